# revision 6
# baseline (speedup 1.0000x reference)
"""CharBiLSTM Trainium2 kernel (v2: fp8 DoubleRow recurrence + host xw tables).

Full inputs in, full output out. Shards the 4096 words across 8 NeuronCores
(data parallel, weights replicated), runs a Bass/Tile kernel per core, and
reassembles the [B, W, H] output on the host.

Strategy (changes vs the 334us fp16 baseline):
  - Input projection is precomputed on the host as a vocab table
    xw[v] = Wih @ emb[v] + (bih + bhh), gathered per char, and streamed to
    the device as an fp16 [128, 16chunk, cols] tensor per direction.  On
    device it enters PSUM through an identity matmul (same PE cost as the
    old Wih @ x matmul) -- but the gate bias rides along for free, which
    makes the PSUM->SBUF activation bias-free so all 4 row-chunks of a gate
    evacuate in ONE merged ScalarE instruction (per-instruction ACT
    overhead was ~30% of ScalarE time).
  - Recurrence matmuls run in fp8e4 (weights pre-scaled by 2^12, h stored
    as 16*h in fp8): steps with >=256 active words use DoubleRow perf mode
    (2 contraction chunks per pass, ~1.5-1.8x), narrower steps use normal
    fp8 (FWL, bf16 rate).  Numerics sim: rel err ~6e-3 vs the 2e-2 gate,
    PROVIDED the projection reads an fp16 (pre-quantization) copy of h --
    so the cell update dual-writes h in fp16 (for the projection) and
    16*h in fp8 (for the next step's matmul).
  - Cell state c in fp16: every DVE cell op runs in 2x mode.
  - ScalarE applies the 2^-16 descale inside the activation (free).
"""

import sys

if '/opt/trn_rl_repo' not in sys.path:
    sys.path.insert(0, '/opt/trn_rl_repo')

import warnings

warnings.filterwarnings('ignore')

import ml_dtypes
import numpy as np

# Problem dims (hardcoded per spec)
B, W, L = 64, 64, 16
E, H, V = 128, 512, 256
N = B * W
N_CORES = 8
KC = H // 128          # Whh contraction chunks
MC = (4 * H) // 128    # gate-row chunks (16); gate g occupies chunks 4g..4g+3
P = 128

DR_MIN = 256           # min block width for DoubleRow mode
W_SCALE = 4096.0       # Whh pre-scale (2^12); max |Whh|*4096 ~ 181 < 240
H_SCALE = 16.0         # h stored as 16*h in fp8e4
XW_SCALE = 65536.0     # xw table pre-scale = W_SCALE * H_SCALE
F8 = ml_dtypes.float8_e4m3  # IEEE e4m3 (max 240) == TRN FP8_EXP4


# ---------------------------------------------------------------------------
# Host-side scheduling
# ---------------------------------------------------------------------------

def _plan(word_lens_flat):
    """Deal words to cores with equalized length profiles.

    Returns (per_core_words, counts):
      per_core_words: [8][M_pad] global word id or -1 for dummy, sorted by
        descending length class.
      counts: c_t = number of words (per core) with len > t, t = 0..L-1.
        Each per-length-class count is padded to a multiple of 4 so every
        c_t is a multiple of 4 (DVE 2x alignment).
    """
    lens = np.asarray(word_lens_flat)
    ids_by_class = {v: [] for v in range(1, L + 1)}
    order = np.argsort(-lens, kind='stable')
    for wid in order:
        ids_by_class[int(lens[wid])].append(int(wid))

    def pad4(x):
        return (x + 3) // 4 * 4

    m_v = {v: pad4((len(ids_by_class[v]) + N_CORES - 1) // N_CORES)
           for v in range(1, L + 1)}

    per_core = [[] for _ in range(N_CORES)]
    rot = 0
    for v in range(L, 0, -1):
        ids = ids_by_class[v]
        buckets = [[] for _ in range(N_CORES)]
        for i, wid in enumerate(ids):
            buckets[(rot + i) % N_CORES].append(wid)
        rot = (rot + len(ids)) % N_CORES
        for k in range(N_CORES):
            b = buckets[k]
            b += [-1] * (m_v[v] - len(b))
            per_core[k].extend(b)

    counts = [sum(m_v[v] for v in range(t + 1, L + 1)) for t in range(L)]
    return per_core, counts


# ---------------------------------------------------------------------------
# Device program
# ---------------------------------------------------------------------------

def _build_program(counts, m_pad, xcols):
    import concourse.bacc as bacc
    import concourse.mybir as mybir
    import concourse.tile as tile

    f32 = mybir.dt.float32
    f16 = mybir.dt.float16
    f8 = mybir.dt.float8e4
    DRM = mybir.MatmulPerfMode.DoubleRow
    SIG = mybir.ActivationFunctionType.Sigmoid
    TANH = mybir.ActivationFunctionType.Tanh
    MUL = mybir.AluOpType.mult

    # per-direction column offsets into the xw stream (fwd: step order,
    # bwd: reverse step order == round order)
    off_f = np.concatenate([[0], np.cumsum(counts)]).astype(int)
    off_b = {t: int(sum(counts[s] for s in range(t + 1, L))) for t in range(L)}

    TW = (m_pad + 15) // 16 * 16  # state tile width (mult of 16)
    XW_W = counts[0]

    nc = bacc.Bacc(None, target_bir_lowering=False)

    xw_d = {d: nc.dram_tensor(f"xw_{d}", [P, MC, xcols], f16, kind="ExternalInput")
            for d in ("f", "b")}
    whh_d = {d: nc.dram_tensor(f"whh_{d}", [P, KC, 4 * H], f8, kind="ExternalInput")
             for d in ("f", "b")}
    ident_d = nc.dram_tensor("ident", [P, P], f16, kind="ExternalInput")
    wp_d = nc.dram_tensor("wp", [P, 8, H], f16, kind="ExternalInput")
    bp_d = nc.dram_tensor("bp", [P, H], f32, kind="ExternalInput")
    out_d = nc.dram_tensor("out", [m_pad, H], f32, kind="ExternalOutput")

    with tile.TileContext(nc) as tc:
        with tc.tile_pool(name="persist", bufs=1) as pp:
            ident = pp.tile([P, P], f16)
            nc.sync.dma_start(out=ident, in_=ident_d[:, :])

            whh = {d: pp.tile([P, KC, 4 * H], f8, name=f"whh{d}") for d in ("f", "b")}
            hS8 = {}
            hS16 = {}
            cS = {}
            for d in ("f", "b"):
                hS8[d] = pp.tile([P, KC, TW], f8, name=f"h8{d}")
                hS16[d] = pp.tile([P, KC, TW], f16, name=f"h16{d}")
                cS[d] = pp.tile([P, KC, TW], f16, name=f"c{d}")
            # fwd first step writes every column; bwd columns enter
            # mid-sequence and must read zeros.
            nc.gpsimd.memset(hS8["b"].bitcast(f32), 0.0)
            nc.gpsimd.memset(cS["b"].bitcast(f32), 0.0)

            wp = pp.tile([P, 8, H], f16)
            bp = pp.tile([P, H], f32)

            def dir_gates(d, t, xt, first=False):
                """Emit the gate matmuls + merged activations; return the
                per-block gate tiles for the cell phase."""
                c = counts[t]
                gates = (((0, 'i'), (2, 'g'), (3, 'o')) if first else
                         ((0, 'i'), (2, 'g'), (1, 'f'), (3, 'o')))
                blocks = [(0, c)] if c <= 512 else [(0, 512), (512, c - 512)]
                out = []
                for (bs, bw) in blocks:
                    use_dr = (not first) and bw >= DR_MIN
                    sb = {}
                    for g_i, gname in gates:
                        gt = gp.tile([P, 4, 512], f16, name=f"sb{gname}",
                                     tag=f"g{gname}", bufs=2)
                        ps = psp.tile([P, 4, 512], f32, name=f"ps{gname}",
                                      tag="ps", bufs=2)
                        for j in range(4):
                            m = g_i * 4 + j
                            ms = slice(m * P, (m + 1) * P)
                            nc.tensor.matmul(
                                ps[:, j, :bw], ident,
                                xt[:, m, bs:bs + bw],
                                start=True, stop=first)
                            if first:
                                continue
                            if use_dr:
                                for kp in range(2):
                                    nc.tensor.matmul(
                                        ps[:, j, :bw],
                                        whh[d][:, 2 * kp:2 * kp + 2, ms],
                                        hS8[d][:, 2 * kp:2 * kp + 2, bs:bs + bw],
                                        start=False, stop=(kp == 1),
                                        perf_mode=DRM)
                            else:
                                for k in range(KC):
                                    nc.tensor.matmul(
                                        ps[:, j, :bw],
                                        whh[d][:, k, ms],
                                        hS8[d][:, k, bs:bs + bw],
                                        start=False, stop=(k == KC - 1))
                        func = TANH if g_i == 2 else SIG
                        nc.scalar.activation(gt[:, :, :bw], ps[:, :, :bw],
                                             func, scale=1.0 / XW_SCALE)
                        sb[gname] = gt
                    out.append((bs, bw, sb))
                return out

            def dir_cell(d, t, gate_blocks, first=False, fine_cols=False):
                """VectorE cell update (all fp16 -> 2x mode); h written twice:
                fp16 (projection reads it) and 16*h fp8 (next step's rhs)."""
                for (bs, bw, sb) in gate_blocks:
                    tc_t = gp.tile([P, 4, 512], f16, name="tanhc", tag="tc",
                                   bufs=2)
                    if fine_cols:
                        edges = [bs]
                        nxt = (bs // P + 1) * P
                        while nxt < bs + bw:
                            edges.append(nxt)
                            nxt += P
                        edges.append(bs + bw)
                        ranges = [(a, b - a) for a, b in zip(edges, edges[1:])]
                    else:
                        ranges = [(bs, bw)]
                    for (rs, rw) in ranges:
                        lo = rs - bs
                        csl = cS[d][:, :, rs:rs + rw]
                        si = sb['i'][:, :, lo:lo + rw]
                        sg = sb['g'][:, :, lo:lo + rw]
                        so = sb['o'][:, :, lo:lo + rw]
                        tcs = tc_t[:, :, lo:lo + rw]
                        if first:
                            nc.vector.tensor_mul(csl, si, sg)
                        else:
                            nc.vector.tensor_mul(si, si, sg)
                            nc.vector.tensor_mul(
                                csl, sb['f'][:, :, lo:lo + rw], csl)
                            nc.vector.tensor_add(csl, csl, si)
                        nc.scalar.activation(tcs, csl, TANH)
                        nc.vector.tensor_mul(
                            hS16[d][:, :, rs:rs + rw], so, tcs)
                        nc.vector.scalar_tensor_tensor(
                            hS8[d][:, :, rs:rs + rw],
                            so, H_SCALE, tcs, MUL, MUL)

            with tc.tile_pool(name="xw", bufs=1) as xwp, \
                 tc.tile_pool(name="gates", bufs=2) as gp, \
                 tc.tile_pool(name="psum", bufs=2, space="PSUM") as psp:
                for r in range(L):
                    tf = r
                    tb = L - 1 - r
                    cf, cb = counts[tf], counts[tb]
                    xtf = xwp.tile([P, MC, XW_W], f16, name=f"xwf{r}",
                                   tag="xwf", bufs=2)
                    xtb = xwp.tile([P, MC, XW_W], f16, name=f"xwb{r}",
                                   tag="xwb", bufs=2)
                    # xw streams: fwd on the sync queue, bwd on gpsimd,
                    # split into gate-use-order pieces (i, g, f, o) so each
                    # round can start on its first gate before the full
                    # tile lands.  Weights ride the scalar queue.
                    xof, xob = off_f[tf], off_b[tb]
                    pieces = (((0, 4), (8, 12), (12, 16)) if r == 0 else
                              ((0, 4), (8, 12), (4, 8), (12, 16)))
                    for (mlo, mhi) in pieces:
                        nc.sync.dma_start(
                            out=xtf[:, mlo:mhi, :cf],
                            in_=xw_d["f"][:, mlo:mhi, xof:xof + cf])
                        nc.gpsimd.dma_start(
                            out=xtb[:, mlo:mhi, :cb],
                            in_=xw_d["b"][:, mlo:mhi, xob:xob + cb])
                    if r == 0:
                        # whh per gate-column block, both dirs interleaved
                        for g_i in (0, 2, 1, 3):
                            gs = slice(g_i * H, (g_i + 1) * H)
                            nc.scalar.dma_start(out=whh["f"][:, :, gs],
                                                in_=whh_d["f"][:, :, gs])
                            nc.scalar.dma_start(out=whh["b"][:, :, gs],
                                                in_=whh_d["b"][:, :, gs])
                    if r == 1:
                        nc.scalar.dma_start(out=wp, in_=wp_d[:, :, :])
                        nc.scalar.dma_start(out=bp, in_=bp_d[:, :])

                    # gates of both directions first, cell updates after:
                    # keeps the second direction's gate ACTs from queueing
                    # behind the first direction's DVE-dependent cell tanh.
                    first = (r == 0)
                    if cf >= cb:
                        gbf = dir_gates('f', tf, xtf, first=first)
                        gbb = dir_gates('b', tb, xtb, first=first)
                        dir_cell('f', tf, gbf, first=first)
                        dir_cell('b', tb, gbb, first=first,
                                 fine_cols=(r == L - 1))
                    else:
                        gbb = dir_gates('b', tb, xtb, first=first)
                        gbf = dir_gates('f', tf, xtf, first=first)
                        dir_cell('b', tb, gbb, first=first,
                                 fine_cols=(r == L - 1))
                        dir_cell('f', tf, gbf, first=first)

            # Projection: out[words, H] = [h_bwd; h_fwd]^T @ Wp^T + bp
            with tc.tile_pool(name="proj", bufs=2) as prp, \
                 tc.tile_pool(name="prps", bufs=2, space="PSUM") as prps:
                nmc = (m_pad + P - 1) // P
                for mi in range(nmc):
                    pw = min(P, m_pad - mi * P)
                    po = prps.tile([P, H], f32, name="po", tag="po")
                    for kk in range(8):
                        src = hS16['b'] if kk < 4 else hS16['f']
                        nc.tensor.matmul(
                            po[:pw, :],
                            src[:, kk % 4, mi * P:mi * P + pw],
                            wp[:, kk, :],
                            start=(kk == 0), stop=(kk == 7))
                    ot = prp.tile([P, H], f32, name="ot", tag="ot")
                    nc.vector.tensor_add(ot[:pw, :], po[:pw, :], bp[:pw, :])
                    nc.sync.dma_start(out=out_d[mi * P:mi * P + pw, :],
                                      in_=ot[:pw, :])

    nc.compile()
    return nc


# ---------------------------------------------------------------------------
# Entry points
# ---------------------------------------------------------------------------

def _run(inputs, trace=False, tmpdir=None, reps=1):
    from concourse.bass_utils import run_bass_kernel_spmd

    chars = np.asarray(inputs["chars"]).reshape(N, L)
    lens = np.asarray(inputs["word_lens"]).reshape(N).astype(np.int64)
    emb = np.asarray(inputs["emb"], dtype=np.float32)

    per_core, counts = _plan(lens)
    m_pad = len(per_core[0])
    ctot = int(sum(counts))
    xcols = ctot

    # xw vocab tables (input projection + gate bias), pre-scaled
    tab16 = {}
    for d in ("f", "b"):
        Wih = np.asarray(inputs[f"Wih_{d}"], np.float32)
        bias = (np.asarray(inputs[f"bih_{d}"], np.float32) +
                np.asarray(inputs[f"bhh_{d}"], np.float32))
        tab16[d] = ((Wih @ emb.T + bias[:, None]) * XW_SCALE).astype(np.float16)

    whh8 = {}
    for d in ("f", "b"):
        Whh = np.asarray(inputs[f"Whh_{d}"], np.float32)
        whh8[d] = np.ascontiguousarray(
            (Whh.T * W_SCALE).reshape(KC, P, 4 * H).transpose(1, 0, 2)).astype(F8)

    Wp = np.asarray(inputs["Wp"], np.float32)
    bp = np.asarray(inputs["bp"], np.float32)
    wp_sb = np.ascontiguousarray(
        Wp.T.reshape(8, P, H).transpose(1, 0, 2)).astype(np.float16)
    bp_sb = np.ascontiguousarray(np.tile(bp[None, :], (P, 1))).astype(np.float32)
    ident = np.eye(P, dtype=np.float16)

    in_maps = []
    for k in range(N_CORES):
        wl = per_core[k]
        cw = np.zeros((m_pad, L), dtype=np.int64)
        for r, wid in enumerate(wl):
            if wid >= 0:
                cw[r] = chars[wid]
        idx_f = np.concatenate([cw[:counts[t], t] for t in range(L)])
        idx_b = np.concatenate([cw[:counts[t], t] for t in range(L - 1, -1, -1)])
        xw = {}
        for d, idx in (("f", idx_f), ("b", idx_b)):
            g = tab16[d][:, idx]                       # [2048, ctot] fp16
            xw[d] = np.ascontiguousarray(
                g.reshape(MC, P, ctot).transpose(1, 0, 2))
        in_maps.append(dict(
            xw_f=xw["f"], xw_b=xw["b"], whh_f=whh8["f"], whh_b=whh8["b"],
            ident=ident, wp=wp_sb, bp=bp_sb))

    nc = _build_program(counts, m_pad, xcols)
    times = []
    for r in range(reps):
        td = (tmpdir + f"_{r}") if (tmpdir and trace) else tmpdir
        res = run_bass_kernel_spmd(nc, in_maps, list(range(N_CORES)),
                                   trace=trace, tmpdir=td)
        times.append(res.exec_time_ns)
    res.all_exec_times = times

    out = np.zeros((N, H), dtype=np.float32)
    for k in range(N_CORES):
        ok = res.results[k]["out"]
        for r, wid in enumerate(per_core[k]):
            if wid >= 0:
                out[wid] = ok[r]
    return out.reshape(B, W, H), res


def kernel(**inputs):
    out, _ = _run(inputs, trace=False)
    return out


# revision 10
# speedup vs baseline: 1.0736x; 1.0736x over previous
"""CharBiLSTM Trainium2 kernel (v2: fp8 DoubleRow recurrence + host xw tables).

Full inputs in, full output out. Shards the 4096 words across 8 NeuronCores
(data parallel, weights replicated), runs a Bass/Tile kernel per core, and
reassembles the [B, W, H] output on the host.

Strategy (changes vs the 334us fp16 baseline):
  - Input projection is precomputed on the host as a vocab table
    xw[v] = Wih @ emb[v] + (bih + bhh), gathered per char, and streamed to
    the device as an fp16 [128, 16chunk, cols] tensor per direction.  On
    device it enters PSUM through an identity matmul (same PE cost as the
    old Wih @ x matmul) -- but the gate bias rides along for free, which
    makes the PSUM->SBUF activation bias-free so all 4 row-chunks of a gate
    evacuate in ONE merged ScalarE instruction (per-instruction ACT
    overhead was ~30% of ScalarE time).
  - Recurrence matmuls run in fp8e4 (weights pre-scaled by 2^12, h stored
    as 16*h in fp8): steps with >=256 active words use DoubleRow perf mode
    (2 contraction chunks per pass, ~1.5-1.8x), narrower steps use normal
    fp8 (FWL, bf16 rate).  Numerics sim: rel err ~6e-3 vs the 2e-2 gate,
    PROVIDED the projection reads an fp16 (pre-quantization) copy of h --
    so the cell update dual-writes h in fp16 (for the projection) and
    16*h in fp8 (for the next step's matmul).
  - Cell state c in fp16: every DVE cell op runs in 2x mode.
  - ScalarE applies the 2^-16 descale inside the activation (free).
"""

import sys

if '/opt/trn_rl_repo' not in sys.path:
    sys.path.insert(0, '/opt/trn_rl_repo')

import warnings

warnings.filterwarnings('ignore')

import ml_dtypes
import numpy as np

# Problem dims (hardcoded per spec)
B, W, L = 64, 64, 16
E, H, V = 128, 512, 256
N = B * W
N_CORES = 8
KC = H // 128          # Whh contraction chunks
MC = (4 * H) // 128    # gate-row chunks (16); gate g occupies chunks 4g..4g+3
P = 128

DR_MIN = 256           # min block width for DoubleRow mode
W_SCALE = 4096.0       # Whh pre-scale (2^12); max |Whh|*4096 ~ 181 < 240
H_SCALE = 16.0         # h stored as 16*h in fp8e4
XW_SCALE = 65536.0     # xw table pre-scale = W_SCALE * H_SCALE
F8 = ml_dtypes.float8_e4m3  # IEEE e4m3 (max 240) == TRN FP8_EXP4


# ---------------------------------------------------------------------------
# Host-side scheduling
# ---------------------------------------------------------------------------

def _plan(word_lens_flat):
    """Deal words to cores with equalized length profiles.

    Returns (per_core_words, counts):
      per_core_words: [8][M_pad] global word id or -1 for dummy, sorted by
        descending length class.
      counts: c_t = number of words (per core) with len > t, t = 0..L-1.
        Each per-length-class count is padded to a multiple of 4 so every
        c_t is a multiple of 4 (DVE 2x alignment).
    """
    lens = np.asarray(word_lens_flat)
    ids_by_class = {v: [] for v in range(1, L + 1)}
    order = np.argsort(-lens, kind='stable')
    for wid in order:
        ids_by_class[int(lens[wid])].append(int(wid))

    def pad2(x):
        return (x + 1) // 2 * 2

    m_v = {v: pad2((len(ids_by_class[v]) + N_CORES - 1) // N_CORES)
           for v in range(1, L + 1)}

    per_core = [[] for _ in range(N_CORES)]
    rot = 0
    for v in range(L, 0, -1):
        ids = ids_by_class[v]
        buckets = [[] for _ in range(N_CORES)]
        for i, wid in enumerate(ids):
            buckets[(rot + i) % N_CORES].append(wid)
        rot = (rot + len(ids)) % N_CORES
        for k in range(N_CORES):
            b = buckets[k]
            b += [-1] * (m_v[v] - len(b))
            per_core[k].extend(b)

    counts = [sum(m_v[v] for v in range(t + 1, L + 1)) for t in range(L)]
    return per_core, counts


# ---------------------------------------------------------------------------
# Device program
# ---------------------------------------------------------------------------

def _build_program(counts, m_pad, xcols):
    import concourse.bacc as bacc
    import concourse.mybir as mybir
    import concourse.tile as tile

    f32 = mybir.dt.float32
    f16 = mybir.dt.float16
    f8 = mybir.dt.float8e4
    DRM = mybir.MatmulPerfMode.DoubleRow
    SIG = mybir.ActivationFunctionType.Sigmoid
    TANH = mybir.ActivationFunctionType.Tanh
    MUL = mybir.AluOpType.mult

    # per-direction column offsets into the xw stream (fwd: step order,
    # bwd: reverse step order == round order)
    off_f = np.concatenate([[0], np.cumsum(counts)]).astype(int)
    off_b = {t: int(sum(counts[s] for s in range(t + 1, L))) for t in range(L)}

    TW = (m_pad + 15) // 16 * 16  # state tile width (mult of 16)
    XW_W = counts[0]

    nc = bacc.Bacc(None, target_bir_lowering=False)

    xw_d = {d: nc.dram_tensor(f"xw_{d}", [P, MC, xcols], f16, kind="ExternalInput")
            for d in ("f", "b")}
    whh_d = {d: nc.dram_tensor(f"whh_{d}", [P, KC, 4 * H], f8, kind="ExternalInput")
             for d in ("f", "b")}
    ident_d = nc.dram_tensor("ident", [P, P], f16, kind="ExternalInput")
    wp_d = nc.dram_tensor("wp", [P, 8, H], f16, kind="ExternalInput")
    bp_d = nc.dram_tensor("bp", [P, H], f32, kind="ExternalInput")
    out_d = nc.dram_tensor("out", [m_pad, H], f32, kind="ExternalOutput")

    with tile.TileContext(nc) as tc:
        with tc.tile_pool(name="persist", bufs=1) as pp:
            ident = pp.tile([P, P], f16)
            nc.sync.dma_start(out=ident, in_=ident_d[:, :])

            whh = {d: pp.tile([P, KC, 4 * H], f8, name=f"whh{d}") for d in ("f", "b")}
            hS8 = {}
            hS16 = {}
            cS = {}
            for d in ("f", "b"):
                hS8[d] = pp.tile([P, KC, TW], f8, name=f"h8{d}")
                hS16[d] = pp.tile([P, KC, TW], f16, name=f"h16{d}")
                cS[d] = pp.tile([P, KC, TW], f16, name=f"c{d}")
            # fwd first step writes every column; bwd columns enter
            # mid-sequence and must read zeros.
            nc.gpsimd.memset(hS8["b"].bitcast(f32), 0.0)
            nc.gpsimd.memset(cS["b"].bitcast(f32), 0.0)

            wp = pp.tile([P, 8, H], f16)
            bp = pp.tile([P, H], f32)

            def dir_gates(d, t, xt, first=False):
                """Emit the gate matmuls + merged activations; return the
                per-block gate tiles for the cell phase."""
                c = counts[t]
                gates = (((0, 'i'), (2, 'g'), (3, 'o')) if first else
                         ((0, 'i'), (2, 'g'), (1, 'f'), (3, 'o')))
                blocks = [(0, c)] if c <= 512 else [(0, 512), (512, c - 512)]
                out = []
                for (bs, bw) in blocks:
                    use_dr = (not first) and bw >= DR_MIN
                    sb = {}
                    for g_i, gname in gates:
                        gt = gp.tile([P, 4, 512], f16, name=f"sb{gname}",
                                     tag=f"g{gname}", bufs=2)
                        ps = psp.tile([P, 4, 512], f32, name=f"ps{gname}",
                                      tag="ps", bufs=2)
                        for j in range(4):
                            m = g_i * 4 + j
                            ms = slice(m * P, (m + 1) * P)
                            nc.tensor.matmul(
                                ps[:, j, :bw], ident,
                                xt[:, m, bs:bs + bw],
                                start=True, stop=first)
                            if first:
                                continue
                            if use_dr:
                                for kp in range(2):
                                    nc.tensor.matmul(
                                        ps[:, j, :bw],
                                        whh[d][:, 2 * kp:2 * kp + 2, ms],
                                        hS8[d][:, 2 * kp:2 * kp + 2, bs:bs + bw],
                                        start=False, stop=(kp == 1),
                                        perf_mode=DRM)
                            else:
                                for k in range(KC):
                                    nc.tensor.matmul(
                                        ps[:, j, :bw],
                                        whh[d][:, k, ms],
                                        hS8[d][:, k, bs:bs + bw],
                                        start=False, stop=(k == KC - 1))
                        func = TANH if g_i == 2 else SIG
                        nc.scalar.activation(gt[:, :, :bw], ps[:, :, :bw],
                                             func, scale=1.0 / XW_SCALE)
                        sb[gname] = gt
                    out.append((bs, bw, sb))
                return out

            def dir_cell(d, t, gate_blocks, first=False, fine_cols=False):
                """VectorE cell update (all fp16 -> 2x mode); h written twice:
                fp16 (projection reads it) and 16*h fp8 (next step's rhs)."""
                for (bs, bw, sb) in gate_blocks:
                    tc_t = gp.tile([P, 4, 512], f16, name="tanhc", tag="tc",
                                   bufs=2)
                    if fine_cols:
                        edges = [bs]
                        nxt = (bs // P + 1) * P
                        while nxt < bs + bw:
                            edges.append(nxt)
                            nxt += P
                        edges.append(bs + bw)
                        ranges = [(a, b - a) for a, b in zip(edges, edges[1:])]
                    else:
                        ranges = [(bs, bw)]
                    # jh halves: the next step's first DoubleRow pair only
                    # needs h8 chunks 0-1, so finishing those first halves
                    # the critical recurrence chain.
                    for (rs, rw) in ranges:
                        lo = rs - bs
                        for jh in (slice(0, 2), slice(2, 4)):
                            csl = cS[d][:, jh, rs:rs + rw]
                            si = sb['i'][:, jh, lo:lo + rw]
                            sg = sb['g'][:, jh, lo:lo + rw]
                            so = sb['o'][:, jh, lo:lo + rw]
                            tcs = tc_t[:, jh, lo:lo + rw]
                            if first:
                                nc.vector.tensor_mul(csl, si, sg)
                            else:
                                nc.vector.tensor_mul(si, si, sg)
                                nc.vector.tensor_mul(
                                    csl, sb['f'][:, jh, lo:lo + rw], csl)
                                nc.vector.tensor_add(csl, csl, si)
                            nc.scalar.activation(tcs, csl, TANH)
                            nc.vector.scalar_tensor_tensor(
                                hS8[d][:, jh, rs:rs + rw],
                                so, H_SCALE, tcs, MUL, MUL)
                            nc.vector.tensor_mul(
                                hS16[d][:, jh, rs:rs + rw], so, tcs)

            with tc.tile_pool(name="xw", bufs=1) as xwp, \
                 tc.tile_pool(name="gates", bufs=2) as gp, \
                 tc.tile_pool(name="psum", bufs=2, space="PSUM") as psp:
                for r in range(L):
                    tf = r
                    tb = L - 1 - r
                    cf, cb = counts[tf], counts[tb]
                    xtf = xwp.tile([P, MC, XW_W], f16, name=f"xwf{r}",
                                   tag="xwf", bufs=2)
                    xtb = xwp.tile([P, MC, XW_W], f16, name=f"xwb{r}",
                                   tag="xwb", bufs=2)
                    # xw streams: fwd on the sync queue, bwd on gpsimd,
                    # split into gate-use-order pieces (i, g, f, o) so each
                    # round can start on its first gate before the full
                    # tile lands.  Weights ride the scalar queue.
                    xof, xob = off_f[tf], off_b[tb]
                    pieces = (((0, 4), (8, 12), (12, 16)) if r == 0 else
                              ((0, 4), (8, 12), (4, 8), (12, 16)))
                    for (mlo, mhi) in pieces:
                        nc.sync.dma_start(
                            out=xtf[:, mlo:mhi, :cf],
                            in_=xw_d["f"][:, mlo:mhi, xof:xof + cf])
                        nc.gpsimd.dma_start(
                            out=xtb[:, mlo:mhi, :cb],
                            in_=xw_d["b"][:, mlo:mhi, xob:xob + cb])
                    if r == 0:
                        # whh for round 1+ on the (early-idle) bwd queue;
                        # dma_start costs the issuing engine ~1.3us, so keep
                        # these off ScalarE.
                        nc.gpsimd.dma_start(out=whh["f"], in_=whh_d["f"][:, :, :])
                        nc.gpsimd.dma_start(out=whh["b"], in_=whh_d["b"][:, :, :])
                    if r == 1:
                        nc.gpsimd.dma_start(out=wp, in_=wp_d[:, :, :])
                        nc.gpsimd.dma_start(out=bp, in_=bp_d[:, :])

                    # gates of both directions first, cell updates after:
                    # keeps the second direction's gate ACTs from queueing
                    # behind the first direction's DVE-dependent cell tanh.
                    first = (r == 0)
                    if cf >= cb:
                        gbf = dir_gates('f', tf, xtf, first=first)
                        gbb = dir_gates('b', tb, xtb, first=first)
                        dir_cell('f', tf, gbf, first=first)
                        dir_cell('b', tb, gbb, first=first,
                                 fine_cols=(r == L - 1))
                    else:
                        gbb = dir_gates('b', tb, xtb, first=first)
                        gbf = dir_gates('f', tf, xtf, first=first)
                        dir_cell('b', tb, gbb, first=first,
                                 fine_cols=(r == L - 1))
                        dir_cell('f', tf, gbf, first=first)

            # Projection: out[words, H] = [h_bwd; h_fwd]^T @ Wp^T + bp
            with tc.tile_pool(name="proj", bufs=2) as prp, \
                 tc.tile_pool(name="prps", bufs=2, space="PSUM") as prps:
                nmc = (m_pad + P - 1) // P
                for mi in range(nmc):
                    pw = min(P, m_pad - mi * P)
                    po = prps.tile([P, H], f32, name="po", tag="po")
                    for kk in range(8):
                        src = hS16['b'] if kk < 4 else hS16['f']
                        nc.tensor.matmul(
                            po[:pw, :],
                            src[:, kk % 4, mi * P:mi * P + pw],
                            wp[:, kk, :],
                            start=(kk == 0), stop=(kk == 7))
                    ot = prp.tile([P, H], f32, name="ot", tag="ot")
                    nc.vector.tensor_add(ot[:pw, :], po[:pw, :], bp[:pw, :])
                    nc.sync.dma_start(out=out_d[mi * P:mi * P + pw, :],
                                      in_=ot[:pw, :])

    nc.compile()
    return nc


# ---------------------------------------------------------------------------
# Entry points
# ---------------------------------------------------------------------------

def _run(inputs, trace=False, tmpdir=None, reps=1):
    from concourse.bass_utils import run_bass_kernel_spmd

    chars = np.asarray(inputs["chars"]).reshape(N, L)
    lens = np.asarray(inputs["word_lens"]).reshape(N).astype(np.int64)
    emb = np.asarray(inputs["emb"], dtype=np.float32)

    per_core, counts = _plan(lens)
    m_pad = len(per_core[0])
    ctot = int(sum(counts))
    xcols = ctot

    # xw vocab tables (input projection + gate bias), pre-scaled
    tab16 = {}
    for d in ("f", "b"):
        Wih = np.asarray(inputs[f"Wih_{d}"], np.float32)
        bias = (np.asarray(inputs[f"bih_{d}"], np.float32) +
                np.asarray(inputs[f"bhh_{d}"], np.float32))
        tab16[d] = ((Wih @ emb.T + bias[:, None]) * XW_SCALE).astype(np.float16)

    whh8 = {}
    for d in ("f", "b"):
        Whh = np.asarray(inputs[f"Whh_{d}"], np.float32)
        whh8[d] = np.ascontiguousarray(
            (Whh.T * W_SCALE).reshape(KC, P, 4 * H).transpose(1, 0, 2)).astype(F8)

    Wp = np.asarray(inputs["Wp"], np.float32)
    bp = np.asarray(inputs["bp"], np.float32)
    wp_sb = np.ascontiguousarray(
        Wp.T.reshape(8, P, H).transpose(1, 0, 2)).astype(np.float16)
    bp_sb = np.ascontiguousarray(np.tile(bp[None, :], (P, 1))).astype(np.float32)
    ident = np.eye(P, dtype=np.float16)

    in_maps = []
    for k in range(N_CORES):
        wl = per_core[k]
        cw = np.zeros((m_pad, L), dtype=np.int64)
        for r, wid in enumerate(wl):
            if wid >= 0:
                cw[r] = chars[wid]
        idx_f = np.concatenate([cw[:counts[t], t] for t in range(L)])
        idx_b = np.concatenate([cw[:counts[t], t] for t in range(L - 1, -1, -1)])
        xw = {}
        for d, idx in (("f", idx_f), ("b", idx_b)):
            g = tab16[d][:, idx]                       # [2048, ctot] fp16
            xw[d] = np.ascontiguousarray(
                g.reshape(MC, P, ctot).transpose(1, 0, 2))
        in_maps.append(dict(
            xw_f=xw["f"], xw_b=xw["b"], whh_f=whh8["f"], whh_b=whh8["b"],
            ident=ident, wp=wp_sb, bp=bp_sb))

    nc = _build_program(counts, m_pad, xcols)
    times = []
    for r in range(reps):
        td = (tmpdir + f"_{r}") if (tmpdir and trace) else tmpdir
        res = run_bass_kernel_spmd(nc, in_maps, list(range(N_CORES)),
                                   trace=trace, tmpdir=td)
        times.append(res.exec_time_ns)
    res.all_exec_times = times

    out = np.zeros((N, H), dtype=np.float32)
    for k in range(N_CORES):
        ok = res.results[k]["out"]
        for r, wid in enumerate(per_core[k]):
            if wid >= 0:
                out[wid] = ok[r]
    return out.reshape(B, W, H), res


def kernel(**inputs):
    out, _ = _run(inputs, trace=False)
    return out
